# revision 1
# baseline (speedup 1.0000x reference)
"""Trainium2 Bass kernel for LowRankOrthogonalProjection.

Reference computes:
    P = W @ W.T                      (D,D) projection onto rank-R subspace
    C = I - P
    out = target @ C.T + source @ P.T

Since P is symmetric and idempotent-free in the formula, this is exactly
    out = target + (source - target) @ W @ W.T
which replaces two (B*S,D)x(D,D) dense matmuls (~0.55 PFLOP) with two
rank-16 matmuls (~2 GFLOP).  The kernel is therefore memory-bound.

Sharding: data-parallel over the flattened (B*S) row dimension across 8
cores; W (tiny) replicated.  No cross-core communication.

Per-core dataflow (rows-per-core RPC = 2048, D = 4096):
  for each 128-row block:
    DMA src (bf16, host-cast) and tgt (f32) tiles in
    DVE:  diff = src - tgt                  -> bf16
    PE :  transpose 128x128 chunks of diff  -> PSUM (bf16)
    ACT:  copy PSUM -> SBUF (diffT)
    PE :  tmpT(16,128)  += Wchunk.T @ diffT  (accumulate over 32 D-chunks)
    ACT:  tmpT -> SBUF bf16
    PE :  corr(128,512) = tmpT.T @ W.T       (8 chunks)
    DVE:  out = corr + tgt                   (f32)
    DMA out
"""

import numpy as np
import ml_dtypes

B, S, D, R = 4, 4096, 4096, 16
N_CORES = 8
ROWS = B * S                 # 16384
RPC = ROWS // N_CORES        # 2048 rows per core
P = 128
DCH = D // P                 # 32 D-chunks

_NC_CACHE = {}


def build_nc(rpc=RPC, reps=1, io_bufs=3, out_bufs=2, out_dma_scalar=False):
    import concourse.bass as bass
    import concourse.mybir as mybir
    import concourse.tile as tile

    bf16 = mybir.dt.bfloat16
    f32 = mybir.dt.float32

    nc = bass.Bass("TRN2", target_bir_lowering=False)

    src = nc.dram_tensor("src", [rpc, D], bf16, kind="ExternalInput")
    tgt = nc.dram_tensor("tgt", [rpc, D], f32, kind="ExternalInput")
    # wc[p, t*R + r] = W[t*128 + p, r]   (W chunked along D for lhsT use)
    wc = nc.dram_tensor("wc", [P, DCH * R], bf16, kind="ExternalInput")
    # wt[r, d] = W[d, r]
    wt = nc.dram_tensor("wt", [R, D], bf16, kind="ExternalInput")
    ident = nc.dram_tensor("ident", [P, P], bf16, kind="ExternalInput")
    out = nc.dram_tensor("out", [rpc, D], f32, kind="ExternalOutput")

    nblk = rpc // P

    with tile.TileContext(nc) as tc:
        with (
            tc.tile_pool(name="const", bufs=1) as cpool,
            tc.tile_pool(name="srcp", bufs=io_bufs) as src_pool,
            tc.tile_pool(name="tgtp", bufs=io_bufs) as tgt_pool,
            tc.tile_pool(name="diffp", bufs=2) as diff_pool,
            tc.tile_pool(name="dtp", bufs=3) as dt_pool,
            tc.tile_pool(name="tmtp", bufs=2) as tmt_pool,
            tc.tile_pool(name="outp", bufs=out_bufs) as out_pool,
            tc.tile_pool(name="ps_t", bufs=3, space="PSUM") as ps_t,
            tc.tile_pool(name="ps_acc", bufs=2, space="PSUM") as ps_acc,
            tc.tile_pool(name="ps_out", bufs=3, space="PSUM") as ps_out,
        ):
            wc_sb = cpool.tile([P, DCH * R], bf16)
            nc.sync.dma_start(wc_sb, wc[:, :])
            wt_sb = cpool.tile([R, D], bf16)
            nc.sync.dma_start(wt_sb, wt[:, :])
            id_sb = cpool.tile([P, P], bf16)
            nc.sync.dma_start(id_sb, ident[:, :])

            for rb in range(nblk * reps):
                rb = rb % nblk
                rs = rb * P
                src_sb = src_pool.tile([P, D], bf16, tag="src")
                tgt_sb = tgt_pool.tile([P, D], f32, tag="tgt")
                nc.sync.dma_start(src_sb, src[rs : rs + P, :])
                nc.sync.dma_start(tgt_sb, tgt[rs : rs + P, :])

                diff_sb = diff_pool.tile([P, D], bf16, tag="diff")
                nc.vector.tensor_sub(diff_sb, src_sb, tgt_sb)

                tmpT_ps = ps_acc.tile([R, P], f32, tag="tmtps")
                for g in range(DCH // 4):
                    tp = ps_t.tile([P, 4 * P], bf16, tag="tps")
                    for j in range(4):
                        t = g * 4 + j
                        nc.tensor.transpose(
                            tp[:, j * P : (j + 1) * P],
                            diff_sb[:, t * P : (t + 1) * P],
                            id_sb,
                        )
                    dT_sb = dt_pool.tile([P, 4 * P], bf16, tag="dT")
                    nc.scalar.copy(dT_sb, tp)
                    for j in range(4):
                        t = g * 4 + j
                        nc.tensor.matmul(
                            tmpT_ps,
                            wc_sb[:, t * R : (t + 1) * R],
                            dT_sb[:, j * P : (j + 1) * P],
                            start=(t == 0),
                            stop=(t == DCH - 1),
                        )

                tmpT_sb = tmt_pool.tile([R, P], bf16, tag="tmt")
                nc.scalar.copy(tmpT_sb, tmpT_ps)

                out_sb = out_pool.tile([P, D], f32, tag="out")
                for g in range(D // 512):
                    op = ps_out.tile([P, 512], f32, tag="ops")
                    nc.tensor.matmul(
                        op,
                        tmpT_sb,
                        wt_sb[:, g * 512 : (g + 1) * 512],
                        start=True,
                        stop=True,
                    )
                    nc.vector.tensor_add(
                        out_sb[:, g * 512 : (g + 1) * 512],
                        op,
                        tgt_sb[:, g * 512 : (g + 1) * 512],
                    )
                out_eng = nc.scalar if out_dma_scalar else nc.sync
                out_eng.dma_start(out[rs : rs + P, :], out_sb)

    return nc


def split_waits(nc, limit=1):
    """Walrus in this toolchain encodes at most one semaphore wait per
    instruction ("Too many sync wait commands").  Tile's scheduler attaches
    up to ~3.  Rewrite: keep the last wait on the instruction and hoist the
    rest onto standalone EventSemaphore instructions (same engine, placed
    immediately before), which is exactly what raw-bass wait_ge emits."""
    import concourse.mybir as mybir

    nsplit = 0
    for fn in nc.m.functions:
        for blk in fn.blocks:
            new = []
            for ins in blk.instructions:
                si = ins.sync_info
                waits = list(si.on_wait) if si is not None and si.on_wait else []
                if len(waits) > limit:
                    for k, w in enumerate(waits[:-limit]):
                        es = mybir.InstEventSemaphore(
                            name=f"{ins.name}-hw{k}",
                            engine=ins.engine,
                            sync_info=mybir.SyncInfo(on_wait=[w], on_update=[]),
                        )
                        new.append(es)
                        nsplit += 1
                    ins.sync_info = mybir.SyncInfo(
                        on_wait=waits[-limit:],
                        on_update=list(si.on_update or []),
                    )
                new.append(ins)
            blk.instructions[:] = new
    return nsplit


def _get_nc(rpc=RPC, reps=1):
    key = (rpc, reps)
    if key not in _NC_CACHE:
        nc = build_nc(rpc, reps)
        nc.finalize()
        split_waits(nc)
        _NC_CACHE[key] = nc
    return _NC_CACHE[key]


def make_host_inputs(source, target, weight):
    """Cast/shard host-side; returns per-core in_maps."""
    bf = ml_dtypes.bfloat16
    src2 = np.ascontiguousarray(source.reshape(ROWS, D)).astype(bf)
    tgt2 = np.ascontiguousarray(target.reshape(ROWS, D))
    if tgt2.dtype != np.float32:
        tgt2 = tgt2.astype(np.float32)
    wc = np.ascontiguousarray(
        weight.reshape(DCH, P, R).transpose(1, 0, 2).reshape(P, DCH * R)
    ).astype(bf)
    wt = np.ascontiguousarray(weight.T).astype(bf)
    ident = np.eye(P, dtype=np.float32).astype(bf)
    in_maps = []
    for c in range(N_CORES):
        sl = slice(c * RPC, (c + 1) * RPC)
        in_maps.append(
            {
                "src": np.ascontiguousarray(src2[sl]),
                "tgt": np.ascontiguousarray(tgt2[sl]),
                "wc": wc,
                "wt": wt,
                "ident": ident,
            }
        )
    return in_maps


# test.py can set this to capture profiling info
LAST_RESULT = None
TRACE = False


def kernel(source, target, weight):
    from concourse.bass_utils import run_bass_kernel_spmd

    global LAST_RESULT
    in_maps = make_host_inputs(
        np.asarray(source), np.asarray(target), np.asarray(weight)
    )
    nc = _get_nc()
    res = run_bass_kernel_spmd(
        nc, in_maps, core_ids=list(range(N_CORES)), trace=TRACE
    )
    LAST_RESULT = res
    outs = [r["out"] for r in res.results]
    full = np.concatenate(outs, axis=0).reshape(B, S, D)
    return np.ascontiguousarray(full.astype(np.float32, copy=False))



# revision 4
# speedup vs baseline: 356.2719x; 356.2719x over previous
"""Trainium2 Bass kernel for LowRankOrthogonalProjection.

Reference computes:
    P = W @ W.T                      (D,D) projection onto rank-R subspace
    C = I - P
    out = target @ C.T + source @ P.T

Since P is symmetric and idempotent-free in the formula, this is exactly
    out = target + (source - target) @ W @ W.T
which replaces two (B*S,D)x(D,D) dense matmuls (~0.55 PFLOP) with two
rank-16 matmuls (~2 GFLOP).  The kernel is therefore memory-bound.

Sharding: data-parallel over the flattened (B*S) row dimension across 8
cores; W (tiny) replicated.  No cross-core communication.

Per-core dataflow (rows-per-core RPC = 2048, D = 4096):
  for each 128-row block:
    DMA src (bf16, host-cast) and tgt (f32) tiles in
    DVE:  diff = src - tgt                  -> bf16
    PE :  transpose 128x128 chunks of diff  -> PSUM (bf16)
    ACT:  copy PSUM -> SBUF (diffT)
    PE :  tmpT(16,128)  += Wchunk.T @ diffT  (accumulate over 32 D-chunks)
    ACT:  tmpT -> SBUF bf16
    PE :  corr(128,512) = tmpT.T @ W.T       (8 chunks)
    DVE:  out = corr + tgt                   (f32)
    DMA out
"""

import contextlib

import numpy as np
import ml_dtypes

B, S, D, R = 4, 4096, 4096, 16
N_CORES = 8
ROWS = B * S                 # 16384
RPC = ROWS // N_CORES        # 2048 rows per core
P = 128
DCH = D // P                 # 32 D-chunks

_NC_CACHE = {}


def build_nc(rpc=RPC, reps=1, loop_n=1, io_bufs=3, out_bufs=2):
    import concourse.bass as bass
    import concourse.mybir as mybir
    import concourse.tile as tile

    bf16 = mybir.dt.bfloat16
    f32 = mybir.dt.float32

    nc = bass.Bass("TRN2", target_bir_lowering=False)

    src = nc.dram_tensor("src", [rpc, D], bf16, kind="ExternalInput")
    tgt = nc.dram_tensor("tgt", [rpc, D], f32, kind="ExternalInput")
    # wc[p, t*R + r] = W[t*128 + p, r]   (W chunked along D for lhsT use)
    wc = nc.dram_tensor("wc", [P, DCH * R], bf16, kind="ExternalInput")
    # wt[r, d] = W[d, r]
    wt = nc.dram_tensor("wt", [R, D], bf16, kind="ExternalInput")
    ident = nc.dram_tensor("ident", [P, P], bf16, kind="ExternalInput")
    out = nc.dram_tensor("out", [rpc, D], f32, kind="ExternalOutput")

    nblk = rpc // P

    with tile.TileContext(nc) as tc:
        with (
            tc.tile_pool(name="const", bufs=1) as cpool,
            tc.tile_pool(name="srcp", bufs=io_bufs) as src_pool,
            tc.tile_pool(name="tgtp", bufs=io_bufs) as tgt_pool,
            tc.tile_pool(name="diffp", bufs=2) as diff_pool,
            tc.tile_pool(name="dtp", bufs=3) as dt_pool,
            tc.tile_pool(name="tmtp", bufs=2) as tmt_pool,
            tc.tile_pool(name="outp", bufs=out_bufs) as out_pool,
            tc.tile_pool(name="ps_t", bufs=3, space="PSUM") as ps_t,
            tc.tile_pool(name="ps_acc", bufs=2, space="PSUM") as ps_acc,
            tc.tile_pool(name="ps_out", bufs=3, space="PSUM") as ps_out,
        ):
            wc_sb = cpool.tile([P, DCH * R], bf16)
            nc.sync.dma_start(wc_sb, wc[:, :])
            wt_sb = cpool.tile([R, D], bf16)
            nc.sync.dma_start(wt_sb, wt[:, :])
            id_sb = cpool.tile([P, P], bf16)
            nc.sync.dma_start(id_sb, ident[:, :])

            loop_cm = tc.For_i(0, loop_n) if loop_n > 1 else contextlib.nullcontext()
            with loop_cm:
                for rb in range(nblk * reps):
                    rb = rb % nblk
                    rs = rb * P
                    src_sb = src_pool.tile([P, D], bf16, tag="src")
                    tgt_sb = tgt_pool.tile([P, D], f32, tag="tgt")
                    nc.sync.dma_start(src_sb, src[rs : rs + P, :])
                    nc.sync.dma_start(tgt_sb, tgt[rs : rs + P, :])

                    diff_sb = diff_pool.tile([P, D], bf16, tag="diff")
                    nc.vector.tensor_sub(diff_sb, src_sb, tgt_sb)

                    tmpT_ps = ps_acc.tile([R, P], f32, tag="tmtps")
                    for g in range(DCH // 4):
                        tp = ps_t.tile([P, 4 * P], bf16, tag="tps")
                        for j in range(4):
                            t = g * 4 + j
                            nc.tensor.transpose(
                                tp[:, j * P : (j + 1) * P],
                                diff_sb[:, t * P : (t + 1) * P],
                                id_sb,
                            )
                        dT_sb = dt_pool.tile([P, 4 * P], bf16, tag="dT")
                        nc.scalar.copy(dT_sb, tp)
                        for j in range(4):
                            t = g * 4 + j
                            nc.tensor.matmul(
                                tmpT_ps,
                                wc_sb[:, t * R : (t + 1) * R],
                                dT_sb[:, j * P : (j + 1) * P],
                                start=(t == 0),
                                stop=(t == DCH - 1),
                            )

                    tmpT_sb = tmt_pool.tile([R, P], bf16, tag="tmt")
                    nc.scalar.copy(tmpT_sb, tmpT_ps)

                    out_sb = out_pool.tile([P, D], f32, tag="out")
                    for g in range(D // 512):
                        op = ps_out.tile([P, 512], f32, tag="ops")
                        nc.tensor.matmul(
                            op,
                            tmpT_sb,
                            wt_sb[:, g * 512 : (g + 1) * 512],
                            start=True,
                            stop=True,
                        )
                        nc.vector.tensor_add(
                            out_sb[:, g * 512 : (g + 1) * 512],
                            op,
                            tgt_sb[:, g * 512 : (g + 1) * 512],
                        )
                    nc.sync.dma_start(out[rs : rs + P, :], out_sb)

    return nc


def split_waits(nc, limit=1):
    """Walrus in this toolchain encodes at most one semaphore wait per
    instruction ("Too many sync wait commands").  Tile's scheduler attaches
    up to ~3.  Rewrite: keep the last wait on the instruction and hoist the
    rest onto standalone EventSemaphore instructions (same engine, placed
    immediately before), which is exactly what raw-bass wait_ge emits."""
    import concourse.mybir as mybir

    nsplit = 0
    for fn in nc.m.functions:
        for blk in fn.blocks:
            new = []
            for ins in blk.instructions:
                si = ins.sync_info
                waits = list(si.on_wait) if si is not None and si.on_wait else []
                if len(waits) > limit:
                    for k, w in enumerate(waits[:-limit]):
                        es = mybir.InstEventSemaphore(
                            name=f"{ins.name}-hw{k}",
                            engine=ins.engine,
                            sync_info=mybir.SyncInfo(on_wait=[w], on_update=[]),
                        )
                        new.append(es)
                        nsplit += 1
                    ins.sync_info = mybir.SyncInfo(
                        on_wait=waits[-limit:],
                        on_update=list(si.on_update or []),
                    )
                new.append(ins)
            blk.instructions[:] = new
    return nsplit


def _get_nc(rpc=RPC, reps=1, loop_n=1):
    key = (rpc, reps, loop_n)
    if key not in _NC_CACHE:
        nc = build_nc(rpc, reps, loop_n)
        nc.finalize()
        split_waits(nc)
        _NC_CACHE[key] = nc
    return _NC_CACHE[key]


def make_host_inputs(source, target, weight):
    """Cast/shard host-side; returns per-core in_maps."""
    bf = ml_dtypes.bfloat16
    src2 = np.ascontiguousarray(source.reshape(ROWS, D)).astype(bf)
    tgt2 = np.ascontiguousarray(target.reshape(ROWS, D))
    if tgt2.dtype != np.float32:
        tgt2 = tgt2.astype(np.float32)
    wc = np.ascontiguousarray(
        weight.reshape(DCH, P, R).transpose(1, 0, 2).reshape(P, DCH * R)
    ).astype(bf)
    wt = np.ascontiguousarray(weight.T).astype(bf)
    ident = np.eye(P, dtype=np.float32).astype(bf)
    in_maps = []
    for c in range(N_CORES):
        sl = slice(c * RPC, (c + 1) * RPC)
        in_maps.append(
            {
                "src": np.ascontiguousarray(src2[sl]),
                "tgt": np.ascontiguousarray(tgt2[sl]),
                "wc": wc,
                "wt": wt,
                "ident": ident,
            }
        )
    return in_maps


# test.py can set this to capture profiling info
LAST_RESULT = None
TRACE = False


def kernel(source, target, weight):
    from concourse.bass_utils import run_bass_kernel_spmd

    global LAST_RESULT
    in_maps = make_host_inputs(
        np.asarray(source), np.asarray(target), np.asarray(weight)
    )
    nc = _get_nc()
    res = run_bass_kernel_spmd(
        nc, in_maps, core_ids=list(range(N_CORES)), trace=TRACE
    )
    LAST_RESULT = res
    outs = [r["out"] for r in res.results]
    full = np.concatenate(outs, axis=0).reshape(B, S, D)
    return np.ascontiguousarray(full.astype(np.float32, copy=False))


# revision 7
# speedup vs baseline: 566.3820x; 1.5897x over previous
"""Trainium2 Bass kernel for LowRankOrthogonalProjection.

Reference computes:
    P = W @ W.T                      (D,D) projection onto rank-R subspace
    C = I - P
    out = target @ C.T + source @ P.T
      == target + (source - target) @ W @ W.T        (P symmetric)

v2 design — transposed dataflow, no on-chip transposes, no diff tensor:
  Host stages per-core srcT (fp8e4, [D, rows]) and tgtT (bf16, [D, rows]);
  the rank-16 projection attenuates src quantization by sqrt(R/D)=1/16,
  so fp8 source costs ~0.2% output error (tolerance 2e-2).

  Phase A (per 128-row D-chunk c):
      DMA srcT_c fp8 + tgtT_c bf16 (tgtT stays resident for phase B)
      PE:  yT[16, rows] += (16W_c)^T srcT_c + (-16W_c)^T tgtT_c   (PSUM f32)
  (so yT = 16 * W^T(src-tgt)^T with zero DVE work)
      ACT: yT -> SBUF bf16
  Phase B (per D-chunk c):
      PE:  corrT_c[128, 512] = (W_c/16) yT-tile                   (PSUM f32)
      j=0,1: DVE fused  outT = corrT(PSUM) + tgtT  -> bf16
      j=2,3: ACT copy corrT->SBUF bf16, GPSIMD add  outT = corr + tgtT
      DMA outT_c out
  Host un-transposes outT -> out (f32).

HBM traffic/core: 8 MB src + 16 MB tgt + 16 MB out = 40 MB (vs 80 MB in v1),
and PE does 3x65k stream columns ~ 82 us < DMA ~112-137 us -> DMA-bound.

Sharding: data-parallel over rows across 8 cores; W replicated; no comms.
"""

import contextlib

import numpy as np
import ml_dtypes

B, S, D, R = 4, 4096, 4096, 16
N_CORES = 8
ROWS = B * S                 # 16384
RPC = ROWS // N_CORES        # 2048 rows per core
P = 128
DCH = D // P                 # 32 D-chunks
NJ = RPC // 512              # 4 row-quarters (PSUM bank = 512 f32)
SCL = 16.0                   # scale W into fp8-normal range; undone in wt

_NC_CACHE = {}


def build_nc(rpc=RPC, reps=1, loop_n=1):
    import concourse.bass as bass
    import concourse.mybir as mybir
    import concourse.tile as tile

    bf16 = mybir.dt.bfloat16
    fp8 = mybir.dt.float8e4
    f32 = mybir.dt.float32

    nc = bass.Bass("TRN2", target_bir_lowering=False)

    srcT = nc.dram_tensor("srcT", [D, rpc], fp8, kind="ExternalInput")
    tgtT = nc.dram_tensor("tgtT", [D, rpc], bf16, kind="ExternalInput")
    # wsrc[p, c*R+r] = SCL*W[c*128+p, r]   (lhsT chunks, fp8, src stream)
    wsrc = nc.dram_tensor("wsrc", [P, DCH * R], fp8, kind="ExternalInput")
    # wneg[p, c*R+r] = -SCL*W[c*128+p, r]  (lhsT chunks, bf16, tgt stream)
    wneg = nc.dram_tensor("wneg", [P, DCH * R], bf16, kind="ExternalInput")
    # wt[r, d] = W[d, r]/SCL               (lhsT slices for corr)
    wt = nc.dram_tensor("wt", [R, D], bf16, kind="ExternalInput")
    outT = nc.dram_tensor("outT", [D, rpc], bf16, kind="ExternalOutput")

    nj = rpc // 512

    with tile.TileContext(nc) as tc:
        with (
            tc.tile_pool(name="const", bufs=1) as cpool,
            tc.tile_pool(name="srcp", bufs=3) as src_pool,
            tc.tile_pool(name="corrp", bufs=4) as corr_pool,
            tc.tile_pool(name="outp", bufs=3) as out_pool,
            tc.tile_pool(name="ps_y", bufs=1, space="PSUM") as ps_y,
            tc.tile_pool(name="ps_c", bufs=4, space="PSUM") as ps_c,
        ):
            wsrc_sb = cpool.tile([P, DCH * R], fp8)
            nc.sync.dma_start(wsrc_sb, wsrc[:, :])
            wneg_sb = cpool.tile([P, DCH * R], bf16)
            nc.sync.dma_start(wneg_sb, wneg[:, :])
            wt_sb = cpool.tile([R, D], bf16)
            nc.sync.dma_start(wt_sb, wt[:, :])
            # resident transposed target (all chunks): 128 x (DCH*rpc) bf16
            tgt_all = cpool.tile([P, DCH * rpc], bf16)
            yt_sb = cpool.tile([R, rpc], bf16)

            loop_cm = tc.For_i(0, loop_n) if loop_n > 1 else contextlib.nullcontext()
            with loop_cm:
                for rep in range(reps):
                    # ---- phase A: accumulate yT = SCL * W^T (src - tgt)^T
                    yts = [
                        ps_y.tile([R, 512], f32, tag=f"yt{j}", name=f"yt{j}")
                        for j in range(nj)
                    ]
                    for c in range(DCH):
                        src_sb = src_pool.tile([P, rpc], fp8, tag="src")
                        nc.sync.dma_start(src_sb, srcT[c * P : (c + 1) * P, :])
                        tslice = tgt_all[:, c * rpc : (c + 1) * rpc]
                        nc.sync.dma_start(tslice, tgtT[c * P : (c + 1) * P, :])
                        for j in range(nj):
                            nc.tensor.matmul(
                                yts[j],
                                wsrc_sb[:, c * R : (c + 1) * R],
                                src_sb[:, j * 512 : (j + 1) * 512],
                                start=(c == 0),
                                stop=False,
                            )
                            nc.tensor.matmul(
                                yts[j],
                                wneg_sb[:, c * R : (c + 1) * R],
                                tslice[:, j * 512 : (j + 1) * 512],
                                start=False,
                                stop=(c == DCH - 1),
                            )
                    for j in range(nj):
                        nc.scalar.copy(yt_sb[:, j * 512 : (j + 1) * 512], yts[j])

                    # ---- phase B: corrT = (W/SCL) yT ; outT = corrT + tgtT
                    for c in range(DCH):
                        out_sb = out_pool.tile([P, rpc], bf16, tag="out")
                        tslice = tgt_all[:, c * rpc : (c + 1) * rpc]
                        for j in range(nj):
                            cps = ps_c.tile([P, 512], f32, tag="cps")
                            nc.tensor.matmul(
                                cps,
                                wt_sb[:, c * P : (c + 1) * P],
                                yt_sb[:, j * 512 : (j + 1) * 512],
                                start=True,
                                stop=True,
                            )
                            osl = out_sb[:, j * 512 : (j + 1) * 512]
                            tsl = tslice[:, j * 512 : (j + 1) * 512]
                            if j < 2:
                                # DVE fused: PSUM + SBUF -> SBUF (1x mode)
                                nc.vector.tensor_add(osl, cps, tsl)
                            else:
                                csb = corr_pool.tile([P, 512], bf16, tag="csb")
                                nc.scalar.copy(csb, cps)
                                nc.gpsimd.tensor_add(osl, csb, tsl)
                        nc.sync.dma_start(outT[c * P : (c + 1) * P, :], out_sb)

    return nc


def split_waits(nc, limit=1):
    """Walrus encodes at most one semaphore wait per instruction.  Hoist
    extra waits onto standalone EventSemaphore instructions."""
    import concourse.mybir as mybir

    nsplit = 0
    for fn in nc.m.functions:
        for blk in fn.blocks:
            new = []
            for ins in blk.instructions:
                si = ins.sync_info
                waits = list(si.on_wait) if si is not None and si.on_wait else []
                if len(waits) > limit:
                    for k, w in enumerate(waits[:-limit]):
                        es = mybir.InstEventSemaphore(
                            name=f"{ins.name}-hw{k}",
                            engine=ins.engine,
                            sync_info=mybir.SyncInfo(on_wait=[w], on_update=[]),
                        )
                        new.append(es)
                        nsplit += 1
                    ins.sync_info = mybir.SyncInfo(
                        on_wait=waits[-limit:],
                        on_update=list(si.on_update or []),
                    )
                new.append(ins)
            blk.instructions[:] = new
    return nsplit


def _get_nc(rpc=RPC, reps=1, loop_n=1):
    key = (rpc, reps, loop_n)
    if key not in _NC_CACHE:
        nc = build_nc(rpc, reps, loop_n)
        nc.finalize()
        split_waits(nc)
        _NC_CACHE[key] = nc
    return _NC_CACHE[key]


def make_host_inputs(source, target, weight):
    """Cast/transpose/shard host-side; returns per-core in_maps."""
    bf = ml_dtypes.bfloat16
    fp8 = ml_dtypes.float8_e4m3
    src2 = source.reshape(ROWS, D)
    tgt2 = target.reshape(ROWS, D)
    w = np.asarray(weight, np.float32)
    ws = (SCL * w).reshape(DCH, P, R).transpose(1, 0, 2).reshape(P, DCH * R)
    wsrc = np.clip(ws, -240, 240).astype(fp8)
    wneg = np.ascontiguousarray(-ws).astype(bf)
    wt = np.ascontiguousarray(w.T / SCL).astype(bf)
    in_maps = []
    for c in range(N_CORES):
        sl = slice(c * RPC, (c + 1) * RPC)
        srcT = np.clip(np.ascontiguousarray(src2[sl].T), -240, 240).astype(fp8)
        tgtT = np.ascontiguousarray(tgt2[sl].T).astype(bf)
        in_maps.append(
            {"srcT": srcT, "tgtT": tgtT, "wsrc": wsrc, "wneg": wneg, "wt": wt}
        )
    return in_maps


# test.py can set this to capture profiling info
LAST_RESULT = None
TRACE = False


def kernel(source, target, weight):
    from concourse.bass_utils import run_bass_kernel_spmd

    global LAST_RESULT
    in_maps = make_host_inputs(
        np.asarray(source), np.asarray(target), np.asarray(weight)
    )
    nc = _get_nc()
    res = run_bass_kernel_spmd(
        nc, in_maps, core_ids=list(range(N_CORES)), trace=TRACE
    )
    LAST_RESULT = res
    out = np.empty((ROWS, D), np.float32)
    for c in range(N_CORES):
        out[c * RPC : (c + 1) * RPC] = res.results[c]["outT"].T.astype(np.float32)
    return out.reshape(B, S, D)


# revision 14
# speedup vs baseline: 644.5189x; 1.1380x over previous
"""Trainium2 Bass kernel for LowRankOrthogonalProjection.

    out = target @ (I - W W^T) + source @ (W W^T)
        = target + (source - target) @ W @ W.T        (P = W W^T symmetric)

v3 design — transposed dataflow, no on-chip transposes, no diff tensor,
row-sub-batch pipelining, 1MB DMA groups, outputs on the ACT HWDGE ring.

  Host stages per-core srcT (fp8e4, [D, rows]) and tgtT (bf16) in
  sub-batch-contiguous blocks; the rank-16 projection attenuates src
  quantization by sqrt(R/D)=1/16, so fp8 source costs ~0.2% output error.

  Per sub-batch s (rows split in NSB blocks):
    Phase A (per 128-row D-chunk c, DMA in 1MB groups):
        PE:  yT[16, rows] += (16W_c)^T srcT_c + (-16W_c)^T tgtT_c  (PSUM)
        ACT: yT -> SBUF bf16
    Phase B (per D-chunk c):
        PE:  corrT_c[128, 512] = (W_c/16) yT-tile                  (PSUM)
        j=0: DVE fused  outT = corrT(PSUM) + tgtT -> bf16
        j=1: ACT copy corrT->SBUF bf16, GPSIMD add
        ACT ring: DMA outT out in 1MB groups (parallel to SP-ring inputs)
  Host un-transposes outT -> out (f32).

HBM traffic/core: 8 MB src + 16 MB tgt + 16 MB out = 40 MB (vs 80 MB v1).
"""

import contextlib

import numpy as np
import ml_dtypes

B, S, D, R = 4, 4096, 4096, 16
N_CORES = 8
ROWS = B * S                 # 16384
RPC = ROWS // N_CORES        # 2048 rows per core
P = 128
DCH = D // P                 # 32 D-chunks
NSB = 2                      # row sub-batches per core
RSB = RPC // NSB             # 1024 rows per sub-batch
SCL = 16.0                   # scale W into fp8-normal range; undone in wt
SRC_G = 8                    # src chunks per DMA group (1 MB)
TGT_G = 4                    # tgt chunks per DMA group (1 MB)
OUT_G = 4                    # out chunks per DMA group (1 MB)

_NC_CACHE = {}


def build_nc(rpc=RPC, reps=1, loop_n=1):
    import concourse.bass as bass
    import concourse.mybir as mybir
    import concourse.tile as tile

    bf16 = mybir.dt.bfloat16
    fp8 = mybir.dt.float8e4
    f32 = mybir.dt.float32

    nc = bass.Bass("TRN2", target_bir_lowering=False)

    nsb = NSB
    rsb = rpc // nsb
    nj = rsb // 512
    nsg = DCH // SRC_G  # src DMA groups per sub-batch
    ntg = DCH // TGT_G  # tgt/out DMA groups per sub-batch

    # All tensors are staged host-side in group-tile layout: each DMA group
    # is a contiguous [128, G*rsb] block matching the SBUF tile exactly
    # (group row p = chunk-major concat of D-rows g*G*128 + gc*128 + p).
    srcT = nc.dram_tensor("srcT", [nsb * nsg * P, SRC_G * rsb], fp8,
                          kind="ExternalInput")
    tgtT = nc.dram_tensor("tgtT", [nsb * ntg * P, TGT_G * rsb], bf16,
                          kind="ExternalInput")
    wsrc = nc.dram_tensor("wsrc", [P, DCH * R], fp8, kind="ExternalInput")
    wneg = nc.dram_tensor("wneg", [P, DCH * R], bf16, kind="ExternalInput")
    wt = nc.dram_tensor("wt", [R, D], bf16, kind="ExternalInput")
    outT = nc.dram_tensor("outT", [nsb * ntg * P, OUT_G * rsb], bf16,
                          kind="ExternalOutput")

    with tile.TileContext(nc) as tc:
        with (
            tc.tile_pool(name="const", bufs=1) as cpool,
            tc.tile_pool(name="tgall", bufs=2) as tgt_pool,
            tc.tile_pool(name="srcp", bufs=2) as src_pool,
            tc.tile_pool(name="corrp", bufs=4) as corr_pool,
            tc.tile_pool(name="outp", bufs=3) as out_pool,
            tc.tile_pool(name="ps_y", bufs=2, space="PSUM") as ps_y,
            tc.tile_pool(name="ps_c", bufs=4, space="PSUM") as ps_c,
        ):
            wsrc_sb = cpool.tile([P, DCH * R], fp8)
            nc.sync.dma_start(wsrc_sb, wsrc[:, :])
            wneg_sb = cpool.tile([P, DCH * R], bf16)
            nc.sync.dma_start(wneg_sb, wneg[:, :])
            wt_sb = cpool.tile([R, D], bf16)
            nc.sync.dma_start(wt_sb, wt[:, :])
            yt_sb = cpool.tile([R, rpc], bf16)

            loop_cm = tc.For_i(0, loop_n) if loop_n > 1 else contextlib.nullcontext()
            with loop_cm:
                for rep in range(reps):
                    for s in range(nsb):
                        # ---- phase A: yT = SCL * W^T (src - tgt)^T
                        tg_all = tgt_pool.tile([P, DCH * rsb], bf16, tag="tg")
                        yts = [
                            ps_y.tile([R, 512], f32, tag=f"yt{j}", name=f"yt{j}")
                            for j in range(nj)
                        ]
                        src_sb = None
                        for c in range(DCH):
                            if c % SRC_G == 0:
                                src_sb = src_pool.tile(
                                    [P, SRC_G * rsb], fp8, tag="src"
                                )
                                r0 = (s * nsg + c // SRC_G) * P
                                nc.sync.dma_start(src_sb, srcT[r0 : r0 + P, :])
                            if c % TGT_G == 0:
                                r0 = (s * ntg + c // TGT_G) * P
                                nc.sync.dma_start(
                                    tg_all[:, c * rsb : (c + TGT_G) * rsb],
                                    tgtT[r0 : r0 + P, :],
                                )
                            co = (c % SRC_G) * rsb
                            for j in range(nj):
                                nc.tensor.matmul(
                                    yts[j],
                                    wsrc_sb[:, c * R : (c + 1) * R],
                                    src_sb[:, co + j * 512 : co + (j + 1) * 512],
                                    start=(c == 0),
                                    stop=False,
                                )
                                nc.tensor.matmul(
                                    yts[j],
                                    wneg_sb[:, c * R : (c + 1) * R],
                                    tg_all[
                                        :,
                                        c * rsb + j * 512 : c * rsb + (j + 1) * 512,
                                    ],
                                    start=False,
                                    stop=(c == DCH - 1),
                                )
                        for j in range(nj):
                            nc.scalar.copy(
                                yt_sb[:, s * rsb + j * 512 : s * rsb + (j + 1) * 512],
                                yts[j],
                            )

                        # ---- phase B: corrT = (W/SCL) yT ; outT = corrT + tgtT
                        out_sb = None
                        pend = None  # (dram_row0, tile) of finished out group
                        for c in range(DCH):
                            if c % OUT_G == 0:
                                out_sb = out_pool.tile(
                                    [P, OUT_G * rsb], bf16, tag="out"
                                )
                            oo = (c % OUT_G) * rsb
                            for j in range(nj):
                                cps = ps_c.tile([P, 512], f32, tag="cps")
                                nc.tensor.matmul(
                                    cps,
                                    wt_sb[:, c * P : (c + 1) * P],
                                    yt_sb[
                                        :,
                                        s * rsb + j * 512 : s * rsb + (j + 1) * 512,
                                    ],
                                    start=True,
                                    stop=True,
                                )
                                osl = out_sb[:, oo + j * 512 : oo + (j + 1) * 512]
                                tsl = tg_all[
                                    :, c * rsb + j * 512 : c * rsb + (j + 1) * 512
                                ]
                                if j == 0:
                                    nc.vector.tensor_add(osl, cps, tsl)
                                else:
                                    csb = corr_pool.tile([P, 512], bf16, tag="csb")
                                    nc.scalar.copy(csb, cps)
                                    nc.gpsimd.tensor_add(osl, csb, tsl)
                            if c % OUT_G == OUT_G - 1:
                                # emit previous group's store now (one group late
                                # so the ACT ring never stalls on fresh adds)
                                if pend is not None:
                                    nc.scalar.dma_start(
                                        outT[pend[0] : pend[0] + P, :], pend[1]
                                    )
                                pend = ((s * ntg + c // OUT_G) * P, out_sb)
                        nc.scalar.dma_start(outT[pend[0] : pend[0] + P, :], pend[1])

    return nc


def split_waits(nc, limit=1):
    """Walrus encodes at most one semaphore wait per instruction.  Hoist
    extra waits onto standalone EventSemaphore instructions."""
    import concourse.mybir as mybir

    nsplit = 0
    for fn in nc.m.functions:
        for blk in fn.blocks:
            new = []
            for ins in blk.instructions:
                si = ins.sync_info
                waits = list(si.on_wait) if si is not None and si.on_wait else []
                if len(waits) > limit:
                    for k, w in enumerate(waits[:-limit]):
                        es = mybir.InstEventSemaphore(
                            name=f"{ins.name}-hw{k}",
                            engine=ins.engine,
                            sync_info=mybir.SyncInfo(on_wait=[w], on_update=[]),
                        )
                        new.append(es)
                        nsplit += 1
                    ins.sync_info = mybir.SyncInfo(
                        on_wait=waits[-limit:],
                        on_update=list(si.on_update or []),
                    )
                new.append(ins)
            blk.instructions[:] = new
    return nsplit


def _get_nc(rpc=RPC, reps=1, loop_n=1):
    key = (rpc, reps, loop_n)
    if key not in _NC_CACHE:
        nc = build_nc(rpc, reps, loop_n)
        nc.finalize()
        split_waits(nc)
        _NC_CACHE[key] = nc
    return _NC_CACHE[key]


def make_host_inputs(source, target, weight):
    """Cast/transpose/shard host-side; returns per-core in_maps."""
    bf = ml_dtypes.bfloat16
    fp8 = ml_dtypes.float8_e4m3
    src2 = source.reshape(ROWS, D)
    tgt2 = target.reshape(ROWS, D)
    w = np.asarray(weight, np.float32)
    ws = (SCL * w).reshape(DCH, P, R).transpose(1, 0, 2).reshape(P, DCH * R)
    wsrc = np.clip(ws, -240, 240).astype(fp8)
    wneg = np.ascontiguousarray(-ws).astype(bf)
    wt = np.ascontiguousarray(w.T / SCL).astype(bf)

    def blocks(a2, sl, dt, G):
        """[D, RPC] transposed slice -> group-tile layout
        [NSB * (DCH//G) * 128, G*RSB]: each DMA group is one contiguous
        [128, G*RSB] tile with free dim (chunk-in-group, row)."""
        at = np.ascontiguousarray(a2[sl].T)  # [D, RPC]
        ng = DCH // G
        # [D, NSB, RSB] view: at[:, s*RSB + i]
        a4 = at.reshape(ng, G, P, NSB, RSB)
        # -> [NSB, ng, P, G, RSB]
        a5 = np.ascontiguousarray(a4.transpose(3, 0, 2, 1, 4))
        a5 = a5.reshape(NSB * ng * P, G * RSB)
        if dt is fp8:
            return np.clip(a5, -240, 240).astype(dt)
        return a5.astype(dt)

    in_maps = []
    for c in range(N_CORES):
        sl = slice(c * RPC, (c + 1) * RPC)
        in_maps.append(
            {
                "srcT": blocks(src2, sl, fp8, SRC_G),
                "tgtT": blocks(tgt2, sl, bf, TGT_G),
                "wsrc": wsrc,
                "wneg": wneg,
                "wt": wt,
            }
        )
    return in_maps


# test.py can set this to capture profiling info
LAST_RESULT = None
TRACE = False


def kernel(source, target, weight):
    from concourse.bass_utils import run_bass_kernel_spmd

    global LAST_RESULT
    in_maps = make_host_inputs(
        np.asarray(source), np.asarray(target), np.asarray(weight)
    )
    nc = _get_nc()
    res = run_bass_kernel_spmd(
        nc, in_maps, core_ids=list(range(N_CORES)), trace=TRACE
    )
    LAST_RESULT = res
    ntg = DCH // OUT_G
    out = np.empty((ROWS, D), np.float32)
    for c in range(N_CORES):
        oT = res.results[c]["outT"]  # [NSB*ntg*P, OUT_G*RSB] group-tile layout
        o5 = oT.reshape(NSB, ntg, P, OUT_G, RSB)
        # -> [NSB, ntg, OUT_G, P, RSB] -> [NSB, D, RSB]
        oD = o5.transpose(0, 1, 3, 2, 4).reshape(NSB, D, RSB)
        for s in range(NSB):
            out[c * RPC + s * RSB : c * RPC + (s + 1) * RSB] = (
                oD[s].T.astype(np.float32)
            )
    return out.reshape(B, S, D)


# revision 17
# speedup vs baseline: 709.7144x; 1.1012x over previous
"""Trainium2 Bass kernel for LowRankOrthogonalProjection.

    out = target @ (I - W W^T) + source @ (W W^T)
        = target + (source - target) @ W @ W.T        (P = W W^T symmetric)

v3 design — transposed dataflow, no on-chip transposes, no diff tensor,
row-sub-batch pipelining, 1MB DMA groups, outputs on the ACT HWDGE ring.

  Host stages per-core srcT (fp8e4, [D, rows]) and tgtT (bf16) in
  sub-batch-contiguous blocks; the rank-16 projection attenuates src
  quantization by sqrt(R/D)=1/16, so fp8 source costs ~0.2% output error.

  Per sub-batch s (rows split in NSB blocks):
    Phase A (per 128-row D-chunk c, DMA in 1MB groups):
        PE:  yT[16, rows] += (16W_c)^T srcT_c + (-16W_c)^T tgtT_c  (PSUM)
        ACT: yT -> SBUF bf16
    Phase B (per D-chunk c):
        PE:  corrT_c[128, 512] = (W_c/16) yT-tile                  (PSUM)
        j=0: DVE fused  outT = corrT(PSUM) + tgtT -> bf16
        j=1: ACT copy corrT->SBUF bf16, GPSIMD add
        ACT ring: DMA outT out in 1MB groups (parallel to SP-ring inputs)
  Host un-transposes outT -> out (f32).

HBM traffic/core: 8 MB src + 16 MB tgt + 16 MB out = 40 MB (vs 80 MB v1).
"""

import contextlib

import numpy as np
import ml_dtypes

B, S, D, R = 4, 4096, 4096, 16
N_CORES = 8
ROWS = B * S                 # 16384
RPC = ROWS // N_CORES        # 2048 rows per core
P = 128
DCH = D // P                 # 32 D-chunks
NSB = 2                      # row sub-batches per core
RSB = RPC // NSB             # 1024 rows per sub-batch
SCL = 16.0                   # scale W into fp8-normal range; undone in wt
SRC_G = 8                    # src chunks per DMA group (1 MB)
TGT_G = 4                    # tgt chunks per DMA group (1 MB)
OUT_G = 4                    # out chunks per DMA group (1 MB)

_NC_CACHE = {}


def build_nc(rpc=RPC, reps=1, loop_n=1, mode="full"):
    import concourse.bass as bass
    import concourse.mybir as mybir
    import concourse.tile as tile

    bf16 = mybir.dt.bfloat16
    fp8 = mybir.dt.float8e4
    f32 = mybir.dt.float32

    nc = bass.Bass("TRN2", target_bir_lowering=False)

    nsb = NSB
    rsb = rpc // nsb
    nj = rsb // 512
    nsg = DCH // SRC_G  # src DMA groups per sub-batch
    ntg = DCH // TGT_G  # tgt/out DMA groups per sub-batch

    # All tensors are staged host-side in group-tile layout: each DMA group
    # is a contiguous [128, G*rsb] block matching the SBUF tile exactly
    # (group row p = chunk-major concat of D-rows g*G*128 + gc*128 + p).
    srcT = nc.dram_tensor("srcT", [nsb * nsg * P, SRC_G * rsb], fp8,
                          kind="ExternalInput")
    tgtT = nc.dram_tensor("tgtT", [nsb * ntg * P, TGT_G * rsb], bf16,
                          kind="ExternalInput")
    wsrc = nc.dram_tensor("wsrc", [P, DCH * R], fp8, kind="ExternalInput")
    wneg = nc.dram_tensor("wneg", [P, DCH * R], bf16, kind="ExternalInput")
    wt = nc.dram_tensor("wt", [R, D], bf16, kind="ExternalInput")
    outT = nc.dram_tensor("outT", [nsb * ntg * P, OUT_G * rsb], bf16,
                          kind="ExternalOutput")

    with tile.TileContext(nc) as tc:
        with (
            tc.tile_pool(name="const", bufs=1) as cpool,
            tc.tile_pool(name="tgall", bufs=2) as tgt_pool,
            tc.tile_pool(name="srcp", bufs=2) as src_pool,
            tc.tile_pool(name="corrp", bufs=4) as corr_pool,
            tc.tile_pool(name="outp", bufs=3) as out_pool,
            tc.tile_pool(name="ps_y", bufs=2, space="PSUM") as ps_y,
            tc.tile_pool(name="ps_c", bufs=4, space="PSUM") as ps_c,
        ):
            wsrc_sb = cpool.tile([P, DCH * R], fp8)
            nc.sync.dma_start(wsrc_sb, wsrc[:, :])
            wneg_sb = cpool.tile([P, DCH * R], bf16)
            nc.sync.dma_start(wneg_sb, wneg[:, :])
            wt_sb = cpool.tile([R, D], bf16)
            nc.sync.dma_start(wt_sb, wt[:, :])
            yt_sb = cpool.tile([R, rpc], bf16)

            loop_cm = tc.For_i(0, loop_n) if loop_n > 1 else contextlib.nullcontext()
            with loop_cm:
                for rep in range(reps):
                    for s in range(nsb):
                        # ---- phase A: yT = SCL * W^T (src - tgt)^T
                        tg_all = tgt_pool.tile([P, DCH * rsb], bf16, tag="tg")
                        yts = [
                            ps_y.tile([R, 512], f32, tag=f"yt{j}", name=f"yt{j}")
                            for j in range(nj)
                        ]
                        src_sb = None
                        for c in range(DCH):
                            if c % SRC_G == 0:
                                src_sb = src_pool.tile(
                                    [P, SRC_G * rsb], fp8, tag="src"
                                )
                                r0 = (s * nsg + c // SRC_G) * P
                                nc.sync.dma_start(src_sb, srcT[r0 : r0 + P, :])
                            if c % TGT_G == 0:
                                r0 = (s * ntg + c // TGT_G) * P
                                nc.sync.dma_start(
                                    tg_all[:, c * rsb : (c + TGT_G) * rsb],
                                    tgtT[r0 : r0 + P, :],
                                )
                            co = (c % SRC_G) * rsb
                            if mode == "dma":
                                continue
                            for j in range(nj):
                                nc.tensor.matmul(
                                    yts[j],
                                    wsrc_sb[:, c * R : (c + 1) * R],
                                    src_sb[:, co + j * 512 : co + (j + 1) * 512],
                                    start=(c == 0),
                                    stop=False,
                                )
                                nc.tensor.matmul(
                                    yts[j],
                                    wneg_sb[:, c * R : (c + 1) * R],
                                    tg_all[
                                        :,
                                        c * rsb + j * 512 : c * rsb + (j + 1) * 512,
                                    ],
                                    start=False,
                                    stop=(c == DCH - 1),
                                )
                        if mode != "dma":
                            for j in range(nj):
                                nc.scalar.copy(
                                    yt_sb[
                                        :, s * rsb + j * 512 : s * rsb + (j + 1) * 512
                                    ],
                                    yts[j],
                                )

                        if mode in ("dma", "aonly"):
                            # store tgt data back as dummy output: same bytes
                            for g in range(ntg):
                                r0 = (s * ntg + g) * P
                                nc.scalar.dma_start(
                                    outT[r0 : r0 + P, :],
                                    tg_all[:, g * OUT_G * rsb : (g + 1) * OUT_G * rsb],
                                )
                            continue

                        # ---- phase B: corrT = (W/SCL) yT ; outT = corrT + tgtT
                        out_sb = None
                        pend = None  # (dram_row0, tile) of finished out group
                        for c in range(DCH):
                            if c % OUT_G == 0:
                                out_sb = out_pool.tile(
                                    [P, OUT_G * rsb], bf16, tag="out"
                                )
                            oo = (c % OUT_G) * rsb
                            for j in range(nj):
                                cps = ps_c.tile([P, 512], f32, tag="cps")
                                nc.tensor.matmul(
                                    cps,
                                    wt_sb[:, c * P : (c + 1) * P],
                                    yt_sb[
                                        :,
                                        s * rsb + j * 512 : s * rsb + (j + 1) * 512,
                                    ],
                                    start=True,
                                    stop=True,
                                )
                                osl = out_sb[:, oo + j * 512 : oo + (j + 1) * 512]
                                tsl = tg_all[
                                    :, c * rsb + j * 512 : c * rsb + (j + 1) * 512
                                ]
                                if j == 0:
                                    nc.vector.tensor_add(osl, cps, tsl)
                                else:
                                    csb = corr_pool.tile([P, 512], bf16, tag="csb")
                                    nc.scalar.copy(csb, cps)
                                    nc.gpsimd.tensor_add(osl, csb, tsl)
                            if c % OUT_G == OUT_G - 1:
                                # emit previous group's store now (one group late
                                # so the ACT ring never stalls on fresh adds)
                                if pend is not None:
                                    nc.scalar.dma_start(
                                        outT[pend[0] : pend[0] + P, :], pend[1]
                                    )
                                pend = ((s * ntg + c // OUT_G) * P, out_sb)
                        nc.scalar.dma_start(outT[pend[0] : pend[0] + P, :], pend[1])

    return nc


def split_waits(nc, limit=1):
    """Walrus encodes at most one semaphore wait per instruction.  Hoist
    extra waits onto standalone EventSemaphore instructions."""
    import concourse.mybir as mybir

    nsplit = 0
    for fn in nc.m.functions:
        for blk in fn.blocks:
            new = []
            for ins in blk.instructions:
                si = ins.sync_info
                waits = list(si.on_wait) if si is not None and si.on_wait else []
                if len(waits) > limit:
                    for k, w in enumerate(waits[:-limit]):
                        es = mybir.InstEventSemaphore(
                            name=f"{ins.name}-hw{k}",
                            engine=ins.engine,
                            sync_info=mybir.SyncInfo(on_wait=[w], on_update=[]),
                        )
                        new.append(es)
                        nsplit += 1
                    ins.sync_info = mybir.SyncInfo(
                        on_wait=waits[-limit:],
                        on_update=list(si.on_update or []),
                    )
                new.append(ins)
            blk.instructions[:] = new
    return nsplit


def _get_nc(rpc=RPC, reps=1, loop_n=1, mode=None):
    import os

    if mode is None:
        mode = os.environ.get("KMODE", "full")
    key = (rpc, reps, loop_n, mode)
    if key not in _NC_CACHE:
        nc = build_nc(rpc, reps, loop_n, mode)
        nc.finalize()
        split_waits(nc)
        _NC_CACHE[key] = nc
    return _NC_CACHE[key]


def make_host_inputs(source, target, weight):
    """Cast/transpose/shard host-side; returns per-core in_maps."""
    bf = ml_dtypes.bfloat16
    fp8 = ml_dtypes.float8_e4m3
    src2 = source.reshape(ROWS, D)
    tgt2 = target.reshape(ROWS, D)
    w = np.asarray(weight, np.float32)
    ws = (SCL * w).reshape(DCH, P, R).transpose(1, 0, 2).reshape(P, DCH * R)
    wsrc = np.clip(ws, -240, 240).astype(fp8)
    wneg = np.ascontiguousarray(-ws).astype(bf)
    wt = np.ascontiguousarray(w.T / SCL).astype(bf)

    def blocks(a2, sl, dt, G):
        """[D, RPC] transposed slice -> group-tile layout
        [NSB * (DCH//G) * 128, G*RSB]: each DMA group is one contiguous
        [128, G*RSB] tile with free dim (chunk-in-group, row)."""
        at = np.ascontiguousarray(a2[sl].T)  # [D, RPC]
        ng = DCH // G
        # [D, NSB, RSB] view: at[:, s*RSB + i]
        a4 = at.reshape(ng, G, P, NSB, RSB)
        # -> [NSB, ng, P, G, RSB]
        a5 = np.ascontiguousarray(a4.transpose(3, 0, 2, 1, 4))
        a5 = a5.reshape(NSB * ng * P, G * RSB)
        if dt is fp8:
            return np.clip(a5, -240, 240).astype(dt)
        return a5.astype(dt)

    in_maps = []
    for c in range(N_CORES):
        sl = slice(c * RPC, (c + 1) * RPC)
        in_maps.append(
            {
                "srcT": blocks(src2, sl, fp8, SRC_G),
                "tgtT": blocks(tgt2, sl, bf, TGT_G),
                "wsrc": wsrc,
                "wneg": wneg,
                "wt": wt,
            }
        )
    return in_maps


# test.py can set this to capture profiling info
LAST_RESULT = None
TRACE = False


def kernel(source, target, weight):
    from concourse.bass_utils import run_bass_kernel_spmd

    global LAST_RESULT
    in_maps = make_host_inputs(
        np.asarray(source), np.asarray(target), np.asarray(weight)
    )
    nc = _get_nc()
    res = run_bass_kernel_spmd(
        nc, in_maps, core_ids=list(range(N_CORES)), trace=TRACE
    )
    LAST_RESULT = res
    ntg = DCH // OUT_G
    out = np.empty((ROWS, D), np.float32)
    for c in range(N_CORES):
        oT = res.results[c]["outT"]  # [NSB*ntg*P, OUT_G*RSB] group-tile layout
        o5 = oT.reshape(NSB, ntg, P, OUT_G, RSB)
        # -> [NSB, ntg, OUT_G, P, RSB] -> [NSB, D, RSB]
        oD = o5.transpose(0, 1, 3, 2, 4).reshape(NSB, D, RSB)
        for s in range(NSB):
            out[c * RPC + s * RSB : c * RPC + (s + 1) * RSB] = (
                oD[s].T.astype(np.float32)
            )
    return out.reshape(B, S, D)


# revision 18
# speedup vs baseline: 927.6396x; 1.3071x over previous
"""Trainium2 Bass kernel for LowRankOrthogonalProjection.

    out = target @ (I - W W^T) + source @ (W W^T)
        = target + (source - target) @ W @ W.T        (P = W W^T symmetric)

v3 design — transposed dataflow, no on-chip transposes, no diff tensor,
row-sub-batch pipelining, 1MB DMA groups, outputs on the ACT HWDGE ring.

  Host stages per-core srcT (fp8e4, [D, rows]) and tgtT (bf16) in
  sub-batch-contiguous blocks; the rank-16 projection attenuates src
  quantization by sqrt(R/D)=1/16, so fp8 source costs ~0.2% output error.

  Per sub-batch s (rows split in NSB blocks):
    Phase A (per 128-row D-chunk c, DMA in 1MB groups):
        PE:  yT[16, rows] += (16W_c)^T srcT_c + (-16W_c)^T tgtT_c  (PSUM)
        ACT: yT -> SBUF bf16
    Phase B (per D-chunk c):
        PE:  corrT_c[128, 512] = (W_c/16) yT-tile                  (PSUM)
        j=0: DVE fused  outT = corrT(PSUM) + tgtT -> bf16
        j=1: ACT copy corrT->SBUF bf16, GPSIMD add
        ACT ring: DMA outT out in 1MB groups (parallel to SP-ring inputs)
  Host un-transposes outT -> out (f32).

HBM traffic/core: 8 MB src + 16 MB tgt + 16 MB out = 40 MB (vs 80 MB v1).
"""

import contextlib

import numpy as np
import ml_dtypes

B, S, D, R = 4, 4096, 4096, 16
N_CORES = 8
ROWS = B * S                 # 16384
RPC = ROWS // N_CORES        # 2048 rows per core
P = 128
DCH = D // P                 # 32 D-chunks
NSB = 2                      # row sub-batches per core
RSB = RPC // NSB             # 1024 rows per sub-batch
SCL = 16.0                   # scale W into fp8-normal range; undone in wt
import os as _os
SRC_G = int(_os.environ.get("KSRCG", "8"))   # src chunks per DMA group (1 MB)
TGT_G = int(_os.environ.get("KTGTG", "4"))   # tgt chunks per DMA group (1 MB)
OUT_G = int(_os.environ.get("KOUTG", "4"))   # out chunks per DMA group (1 MB)

_NC_CACHE = {}


def build_nc(rpc=RPC, reps=1, loop_n=1, mode="full"):
    import concourse.bass as bass
    import concourse.mybir as mybir
    import concourse.tile as tile

    bf16 = mybir.dt.bfloat16
    fp8 = mybir.dt.float8e4
    f32 = mybir.dt.float32

    nc = bass.Bass("TRN2", target_bir_lowering=False)

    nsb = NSB
    rsb = rpc // nsb
    nj = rsb // 512
    nsg = DCH // SRC_G  # src DMA groups per sub-batch
    ntg = DCH // TGT_G  # tgt/out DMA groups per sub-batch

    # All tensors are staged host-side in group-tile layout: each DMA group
    # is a contiguous [128, G*rsb] block matching the SBUF tile exactly
    # (group row p = chunk-major concat of D-rows g*G*128 + gc*128 + p).
    srcT = nc.dram_tensor("srcT", [nsb * nsg * P, SRC_G * rsb], fp8,
                          kind="ExternalInput")
    tgtT = nc.dram_tensor("tgtT", [nsb * ntg * P, TGT_G * rsb], bf16,
                          kind="ExternalInput")
    wsrc = nc.dram_tensor("wsrc", [P, DCH * R], fp8, kind="ExternalInput")
    wneg = nc.dram_tensor("wneg", [P, DCH * R], bf16, kind="ExternalInput")
    wt = nc.dram_tensor("wt", [R, D], bf16, kind="ExternalInput")
    outT = nc.dram_tensor("outT", [nsb * ntg * P, OUT_G * rsb], bf16,
                          kind="ExternalOutput")

    with tile.TileContext(nc) as tc:
        with (
            tc.tile_pool(name="const", bufs=1) as cpool,
            tc.tile_pool(name="tgall", bufs=2) as tgt_pool,
            tc.tile_pool(name="srcp", bufs=2) as src_pool,
            tc.tile_pool(name="corrp", bufs=4) as corr_pool,
            tc.tile_pool(name="outp", bufs=3) as out_pool,
            tc.tile_pool(name="ps_y", bufs=2, space="PSUM") as ps_y,
            tc.tile_pool(name="ps_c", bufs=4, space="PSUM") as ps_c,
        ):
            wsrc_sb = cpool.tile([P, DCH * R], fp8)
            nc.sync.dma_start(wsrc_sb, wsrc[:, :])
            wneg_sb = cpool.tile([P, DCH * R], bf16)
            nc.sync.dma_start(wneg_sb, wneg[:, :])
            wt_sb = cpool.tile([R, D], bf16)
            nc.sync.dma_start(wt_sb, wt[:, :])
            yt_sb = cpool.tile([R, rpc], bf16)

            loop_cm = tc.For_i(0, loop_n) if loop_n > 1 else contextlib.nullcontext()
            with loop_cm:
                for rep in range(reps):
                    for s in range(nsb):
                        # ---- phase A: yT = SCL * W^T (src - tgt)^T
                        tg_all = tgt_pool.tile([P, DCH * rsb], bf16, tag="tg")
                        yts = [
                            ps_y.tile([R, 512], f32, tag=f"yt{j}", name=f"yt{j}")
                            for j in range(nj)
                        ]
                        src_sb = None
                        for c in range(DCH):
                            if c % SRC_G == 0:
                                src_sb = src_pool.tile(
                                    [P, SRC_G * rsb], fp8, tag="src"
                                )
                                r0 = (s * nsg + c // SRC_G) * P
                                nc.sync.dma_start(src_sb, srcT[r0 : r0 + P, :])
                            if c % TGT_G == 0:
                                r0 = (s * ntg + c // TGT_G) * P
                                nc.sync.dma_start(
                                    tg_all[:, c * rsb : (c + TGT_G) * rsb],
                                    tgtT[r0 : r0 + P, :],
                                )
                            co = (c % SRC_G) * rsb
                            if mode == "dma":
                                continue
                            for j in range(nj):
                                nc.tensor.matmul(
                                    yts[j],
                                    wsrc_sb[:, c * R : (c + 1) * R],
                                    src_sb[:, co + j * 512 : co + (j + 1) * 512],
                                    start=(c == 0),
                                    stop=False,
                                )
                                nc.tensor.matmul(
                                    yts[j],
                                    wneg_sb[:, c * R : (c + 1) * R],
                                    tg_all[
                                        :,
                                        c * rsb + j * 512 : c * rsb + (j + 1) * 512,
                                    ],
                                    start=False,
                                    stop=(c == DCH - 1),
                                )
                        if mode != "dma":
                            for j in range(nj):
                                nc.scalar.copy(
                                    yt_sb[
                                        :, s * rsb + j * 512 : s * rsb + (j + 1) * 512
                                    ],
                                    yts[j],
                                )

                        if mode in ("dma", "aonly"):
                            # store tgt data back as dummy output: same bytes
                            for g in range(ntg):
                                r0 = (s * ntg + g) * P
                                nc.scalar.dma_start(
                                    outT[r0 : r0 + P, :],
                                    tg_all[:, g * OUT_G * rsb : (g + 1) * OUT_G * rsb],
                                )
                            continue

                        # ---- phase B: corrT = (W/SCL) yT ; outT = corrT + tgtT
                        out_sb = None
                        pend = None  # (dram_row0, tile) of finished out group
                        for c in range(DCH):
                            if c % OUT_G == 0:
                                out_sb = out_pool.tile(
                                    [P, OUT_G * rsb], bf16, tag="out"
                                )
                            oo = (c % OUT_G) * rsb
                            for j in range(nj):
                                cps = ps_c.tile([P, 512], f32, tag="cps")
                                nc.tensor.matmul(
                                    cps,
                                    wt_sb[:, c * P : (c + 1) * P],
                                    yt_sb[
                                        :,
                                        s * rsb + j * 512 : s * rsb + (j + 1) * 512,
                                    ],
                                    start=True,
                                    stop=True,
                                )
                                osl = out_sb[:, oo + j * 512 : oo + (j + 1) * 512]
                                tsl = tg_all[
                                    :, c * rsb + j * 512 : c * rsb + (j + 1) * 512
                                ]
                                if j == 0:
                                    nc.vector.tensor_add(osl, cps, tsl)
                                else:
                                    csb = corr_pool.tile([P, 512], bf16, tag="csb")
                                    nc.scalar.copy(csb, cps)
                                    nc.gpsimd.tensor_add(osl, csb, tsl)
                            if c % OUT_G == OUT_G - 1:
                                # emit previous group's store now (one group late
                                # so the ACT ring never stalls on fresh adds)
                                if pend is not None:
                                    nc.scalar.dma_start(
                                        outT[pend[0] : pend[0] + P, :], pend[1]
                                    )
                                pend = ((s * ntg + c // OUT_G) * P, out_sb)
                        nc.scalar.dma_start(outT[pend[0] : pend[0] + P, :], pend[1])

    return nc


def split_waits(nc, limit=1):
    """Walrus encodes at most one semaphore wait per instruction.  Hoist
    extra waits onto standalone EventSemaphore instructions."""
    import concourse.mybir as mybir

    nsplit = 0
    for fn in nc.m.functions:
        for blk in fn.blocks:
            new = []
            for ins in blk.instructions:
                si = ins.sync_info
                waits = list(si.on_wait) if si is not None and si.on_wait else []
                if len(waits) > limit:
                    for k, w in enumerate(waits[:-limit]):
                        es = mybir.InstEventSemaphore(
                            name=f"{ins.name}-hw{k}",
                            engine=ins.engine,
                            sync_info=mybir.SyncInfo(on_wait=[w], on_update=[]),
                        )
                        new.append(es)
                        nsplit += 1
                    ins.sync_info = mybir.SyncInfo(
                        on_wait=waits[-limit:],
                        on_update=list(si.on_update or []),
                    )
                new.append(ins)
            blk.instructions[:] = new
    return nsplit


def _get_nc(rpc=RPC, reps=1, loop_n=1, mode=None):
    import os

    if mode is None:
        mode = os.environ.get("KMODE", "full")
    key = (rpc, reps, loop_n, mode)
    if key not in _NC_CACHE:
        nc = build_nc(rpc, reps, loop_n, mode)
        nc.finalize()
        split_waits(nc)
        _NC_CACHE[key] = nc
    return _NC_CACHE[key]


def make_host_inputs(source, target, weight):
    """Cast/transpose/shard host-side; returns per-core in_maps."""
    bf = ml_dtypes.bfloat16
    fp8 = ml_dtypes.float8_e4m3
    src2 = source.reshape(ROWS, D)
    tgt2 = target.reshape(ROWS, D)
    w = np.asarray(weight, np.float32)
    ws = (SCL * w).reshape(DCH, P, R).transpose(1, 0, 2).reshape(P, DCH * R)
    wsrc = np.clip(ws, -240, 240).astype(fp8)
    wneg = np.ascontiguousarray(-ws).astype(bf)
    wt = np.ascontiguousarray(w.T / SCL).astype(bf)

    def blocks(a2, sl, dt, G):
        """[D, RPC] transposed slice -> group-tile layout
        [NSB * (DCH//G) * 128, G*RSB]: each DMA group is one contiguous
        [128, G*RSB] tile with free dim (chunk-in-group, row)."""
        at = np.ascontiguousarray(a2[sl].T)  # [D, RPC]
        ng = DCH // G
        # [D, NSB, RSB] view: at[:, s*RSB + i]
        a4 = at.reshape(ng, G, P, NSB, RSB)
        # -> [NSB, ng, P, G, RSB]
        a5 = np.ascontiguousarray(a4.transpose(3, 0, 2, 1, 4))
        a5 = a5.reshape(NSB * ng * P, G * RSB)
        if dt is fp8:
            return np.clip(a5, -240, 240).astype(dt)
        return a5.astype(dt)

    in_maps = []
    for c in range(N_CORES):
        sl = slice(c * RPC, (c + 1) * RPC)
        in_maps.append(
            {
                "srcT": blocks(src2, sl, fp8, SRC_G),
                "tgtT": blocks(tgt2, sl, bf, TGT_G),
                "wsrc": wsrc,
                "wneg": wneg,
                "wt": wt,
            }
        )
    return in_maps


# test.py can set this to capture profiling info
LAST_RESULT = None
TRACE = False


def kernel(source, target, weight):
    from concourse.bass_utils import run_bass_kernel_spmd

    global LAST_RESULT
    in_maps = make_host_inputs(
        np.asarray(source), np.asarray(target), np.asarray(weight)
    )
    nc = _get_nc()
    res = run_bass_kernel_spmd(
        nc, in_maps, core_ids=list(range(N_CORES)), trace=TRACE
    )
    LAST_RESULT = res
    ntg = DCH // OUT_G
    out = np.empty((ROWS, D), np.float32)
    for c in range(N_CORES):
        oT = res.results[c]["outT"]  # [NSB*ntg*P, OUT_G*RSB] group-tile layout
        o5 = oT.reshape(NSB, ntg, P, OUT_G, RSB)
        # -> [NSB, ntg, OUT_G, P, RSB] -> [NSB, D, RSB]
        oD = o5.transpose(0, 1, 3, 2, 4).reshape(NSB, D, RSB)
        for s in range(NSB):
            out[c * RPC + s * RSB : c * RPC + (s + 1) * RSB] = (
                oD[s].T.astype(np.float32)
            )
    return out.reshape(B, S, D)
